# revision 26
# baseline (speedup 1.0000x reference)
"""Trainium2 Bass kernel for nn_MetaBaseline (global-cosine + DN4 few-shot scoring).

Math (per episode b):
  global: logits[q,k] = <qmean_hat, bmean_hat>          (means over the 5x5 spatial grid)
  DN4:    sim[q,p,k,l] = <q_patch[q,p], s_col_hat[k,l]>  -> sum of top-neighbor_k over l,
          summed over p, / neighbor_k
  out = r0 * logits + r1 * dn4

Device strategy (data-parallel, 8 episodes per NeuronCore):
  - host pre-normalizes the support side and appends the 5 class-mean columns:
    s_ext [640, 130] per episode; query laid out as q_mat [640, 1920] (qp-major,
    zero-padded from 1875); both bf16.
  - host normalizes the query patches too (q_hat), so the device does no scaling;
    the class-mean projections (cols 125:130) then carry a spurious 1/||q_patch||
    factor that the host-built A*||q_patch|| aggregation matrix undoes.
  - PE: sim_ext[qp, 0:130] = q_hat^T @ s_ext as 15 qp-tiles x 5 k-tiles of
    [128,128]x[128,130] bf16 matmuls; two qp-tiles share one fp32 PSUM bank
    [128,260] so each PSUM->SBUF copy (split between ACT and DVE) covers two.
  - DVE Max8 gives the top-8 of each 25-value support-patch group in one op;
    one strided reduce_sum of the first neighbor_k per episode gives the
    per-(patch,class) DN4 terms.
  - tiny matmuls against the aggregation matrices contract the 25 patches of
    each query across partitions (DN4 against the 0/1 matrix, globals against
    A*||q_patch||).
  - host applies 1/(25*||q_mean||), neighbor_k, and the r-weighted combine.
"""
import numpy as np
import ml_dtypes

N_CORES = 8
B, WAY, SHOT, D, H, W = 64, 5, 1, 640, 5, 5
NQ = 75
HW = H * W                 # 25
QP = NQ * HW               # 1875 query patches per episode
NT = 15                    # qp tiles of 128
QP_PAD = NT * 128          # 1920
ND = D // 128              # 5 contraction tiles
EPC = B // N_CORES         # 8 episodes per core
SCOLS = WAY * HW + WAY     # 130
GEPS = 1e-12               # eps of the global-cosine branch (torch F.normalize)

_CACHE = {}
_LAST_IN_MAPS = None


def _build(k: int):
    """Build + compile the SPMD NEFF for top-k = k (k <= 8)."""
    import concourse.bacc as bacc
    import concourse.mybir as mybir
    import concourse.tile as tile

    bf16 = mybir.dt.bfloat16
    f32 = mybir.dt.float32
    COPY = mybir.ActivationFunctionType.Copy

    nc = bacc.Bacc("TRN2", target_bir_lowering=False, debug=False)
    qm = nc.dram_tensor("qm", [EPC, ND, 128, QP_PAD], bf16, kind="ExternalInput")
    se = nc.dram_tensor("se", [ND, 128, EPC * SCOLS], bf16, kind="ExternalInput")
    amat = nc.dram_tensor("amat", [128, NT * NQ], bf16, kind="ExternalInput")
    am2 = nc.dram_tensor("am2", [128, EPC * NT * NQ], bf16, kind="ExternalInput")
    out = nc.dram_tensor("out", [EPC, WAY, 2 * NQ], f32, kind="ExternalOutput")

    with tile.TileContext(nc) as tc:
        with (
            tc.tile_pool(name="const", bufs=1) as cpool,
            tc.tile_pool(name="q", bufs=4 * ND) as qpool,
            tc.tile_pool(name="simps", bufs=4, space="PSUM") as simpool,
            tc.tile_pool(name="acc", bufs=2, space="PSUM") as accpool,
            tc.tile_pool(name="simsb", bufs=14) as sbpool,
            tc.tile_pool(name="out8", bufs=3) as o8pool,
            tc.tile_pool(name="draw", bufs=3) as drpool,
            tc.tile_pool(name="osb", bufs=2) as opool,
        ):
            sts = []
            for d in range(ND):
                st = cpool.tile([128, EPC * SCOLS], bf16, tag=f"se{d}")
                nc.sync.dma_start(st[:, 0:SCOLS], se[d, :, 0:SCOLS])
                sts.append(st)
            amat_t = cpool.tile([128, NT * NQ], bf16)
            am2_t = cpool.tile([128, EPC * NT * NQ], bf16)

            pending = []  # deferred tail: (e, draw, dn4_ps, glob_ps)

            def emit_tail():
                if not pending:
                    return
                e, draw, dn4_ps, glob_ps, simsbs = pending.pop()
                if e != EPC - 1:
                    for t in range(NT):
                        simsb, off = simsbs[t]
                        nc.tensor.matmul(
                            glob_ps[:], simsb[:, off + WAY * HW:off + SCOLS],
                            am2_t[:, (e * NT + t) * NQ:(e * NT + t + 1) * NQ],
                            start=(t == 0), stop=(t == NT - 1),
                        )
                for t in range(NT):
                    nc.tensor.matmul(
                        dn4_ps[:], draw[:, t * WAY:(t + 1) * WAY],
                        amat_t[:, t * NQ:(t + 1) * NQ],
                        start=(t == 0), stop=(t == NT - 1),
                    )
                osb = opool.tile([WAY, 2 * NQ], f32)
                nc.scalar.activation(osb[:, 0:NQ], dn4_ps[:], COPY)
                nc.scalar.activation(osb[:, NQ:2 * NQ], glob_ps[:], COPY)
                nc.sync.dma_start(out[e], osb[:])

            for e in range(EPC):
                qts = []
                for d in range(ND):
                    qt = qpool.tile([128, QP_PAD], bf16)
                    if e == 0:
                        nc.sync.dma_start(qt[:, 0:256], qm[e, d, :, 0:256])
                    else:
                        nc.sync.dma_start(qt[:], qm[e, d])
                    qts.append(qt)
                if e == 0:
                    # tails of the head-chunked tiles, then the big constants
                    for d in range(ND):
                        nc.sync.dma_start(
                            qts[d][:, 256:QP_PAD], qm[e, d, :, 256:QP_PAD])
                        nc.sync.dma_start(
                            sts[d][:, SCOLS:EPC * SCOLS], se[d, :, SCOLS:EPC * SCOLS])
                    nc.sync.dma_start(amat_t[:], amat[:])
                    nc.sync.dma_start(am2_t[:], am2[:])
                dn4_ps = accpool.tile([WAY, NQ], f32, tag="dn4ps")
                glob_ps = accpool.tile([WAY, NQ], f32, tag="globps")
                out8 = o8pool.tile([128, NT * WAY * 8], bf16)
                # tiles paired two-per-PSUM-bank: [0,1], [2,3], ..., [14]
                groups = [(2 * i, min(2 * i + 2, NT)) for i in range((NT + 1) // 2)]
                simsbs = {}
                for gi, (t0, t1) in enumerate(groups):
                    w = (t1 - t0) * SCOLS
                    simps = simpool.tile([128, 2 * SCOLS], f32, tag="simps")
                    for t in range(t0, t1):
                        off = (t - t0) * SCOLS
                        for d in range(ND):
                            nc.tensor.matmul(
                                simps[:, off:off + SCOLS],
                                qts[d][:, t * 128:(t + 1) * 128],
                                sts[d][:, e * SCOLS:(e + 1) * SCOLS],
                                start=(d == 0), stop=(d == ND - 1),
                            )
                    simsb = sbpool.tile([128, 2 * SCOLS], bf16)
                    for t in range(t0, t1):
                        off = (t - t0) * SCOLS
                        if gi == 0:
                            nc.vector.tensor_copy(
                                simsb[:, off:off + SCOLS], simps[:, off:off + SCOLS])
                        else:
                            nc.scalar.activation(
                                simsb[:, off:off + SCOLS], simps[:, off:off + SCOLS], COPY)
                        simsbs[t] = (simsb, off)
                        for kk in range(WAY):
                            g = t * WAY + kk
                            nc.vector.max(
                                out8[:, g * 8:(g + 1) * 8],
                                simsb[:, off + kk * HW:off + (kk + 1) * HW],
                            )
                    if gi == 1:
                        emit_tail()  # previous episode's aggregation matmuls
                    if e == EPC - 1:
                        for t in range(t0, t1):
                            simsb, off = simsbs[t]
                            nc.tensor.matmul(
                                glob_ps[:], simsb[:, off + WAY * HW:off + SCOLS],
                                am2_t[:, (e * NT + t) * NQ:(e * NT + t + 1) * NQ],
                                start=(t == 0), stop=(t == NT - 1),
                            )
                draw = drpool.tile([128, NT * WAY], bf16)
                o8v = out8[:].rearrange("p (g e) -> p g e", e=8)[:, :, 0:k]
                with nc.allow_low_precision("bf16 top-k sums feed a bf16 matmul"):
                    nc.vector.reduce_sum(draw[:], o8v, axis=mybir.AxisListType.X)
                pending.append((e, draw, dn4_ps, glob_ps, simsbs))
            emit_tail()
    nc.compile()
    return nc


def kernel(base, query, r, neighbor_k):
    from concourse.bass_utils import run_bass_kernel_spmd

    k = int(neighbor_k)
    assert 1 <= k <= 8, f"top-k must fit the Max8 output, got {k}"
    base = np.asarray(base, dtype=np.float32).reshape(B, WAY, D, HW)
    query = np.asarray(query, dtype=np.float32).reshape(B, NQ, D, HW)
    r = np.asarray(r, dtype=np.float32)

    # ---- host prep (layout + normalization metadata) ----
    # support: normalized columns + normalized class means -> s_ext [B, D, 130]
    s_norm = base / np.linalg.norm(base, axis=2, keepdims=True)
    bmean = base.mean(axis=3)                                     # [B, way, D]
    bm = bmean / np.maximum(
        np.linalg.norm(bmean, axis=2, keepdims=True), GEPS)
    s_ext = np.empty((B, D, SCOLS), dtype=np.float32)
    s_ext[:, :, :WAY * HW] = s_norm.transpose(0, 2, 1, 3).reshape(B, D, WAY * HW)
    s_ext[:, :, WAY * HW:] = bm.transpose(0, 2, 1)
    # [B, ND, 128, SCOLS] -> per-core [ND, 128, EPC*SCOLS]
    s_ext = s_ext.reshape(B, ND, 128, SCOLS).astype(ml_dtypes.bfloat16)
    s_ext = s_ext.reshape(N_CORES, EPC, ND, 128, SCOLS).transpose(0, 2, 3, 1, 4)
    s_ext = s_ext.reshape(N_CORES, ND, 128, EPC * SCOLS)

    # query: normalized patches, q_hat [B, D, 1920] (qp-major, zero-padded), bf16
    qn = np.sqrt(np.einsum("bqdp,bqdp->bqp", query, query))      # [B, nq, hw]
    q_hat = query / qn[:, :, None, :]
    q_mat = np.zeros((B, D, QP_PAD), dtype=ml_dtypes.bfloat16)
    q_mat[:, :, :QP] = q_hat.transpose(0, 2, 1, 3).reshape(B, D, QP)
    q_mat = q_mat.reshape(B, ND, 128, QP_PAD)
    qn_pad = np.zeros((B, QP_PAD), dtype=np.float32)
    qn_pad[:, :QP] = qn.reshape(B, QP)

    # query-mean norms for the global branch
    qmean = query.mean(axis=3)                                    # [B, nq, D]
    qmn = np.maximum(np.linalg.norm(qmean, axis=2), GEPS)         # [B, nq]

    # patch->query aggregation matrix (0/1), [128, NT*NQ]; and A*||q_patch||
    am = np.zeros((128, NT, NQ), dtype=np.float32)
    for t in range(NT):
        qp_idx = t * 128 + np.arange(128)
        valid = qp_idx < QP
        am[valid, t, qp_idx[valid] // HW] = 1.0
    am2 = am[None] * qn_pad.reshape(B, NT, 128).transpose(0, 2, 1)[:, :, :, None]
    am = am.reshape(128, NT * NQ).astype(ml_dtypes.bfloat16)
    am2 = am2.reshape(N_CORES, EPC, 128, NT * NQ).transpose(0, 2, 1, 3)
    am2 = np.ascontiguousarray(am2.reshape(N_CORES, 128, EPC * NT * NQ)).astype(ml_dtypes.bfloat16)

    if k not in _CACHE:
        _CACHE[k] = _build(k)
    nc = _CACHE[k]

    in_maps = []
    for c in range(N_CORES):
        sl = slice(c * EPC, (c + 1) * EPC)
        in_maps.append({
            "qm": np.ascontiguousarray(q_mat[sl]),
            "se": np.ascontiguousarray(s_ext[c]),
            "amat": am,
            "am2": am2[c],
        })
    global _LAST_IN_MAPS
    _LAST_IN_MAPS = in_maps
    res = run_bass_kernel_spmd(nc, in_maps, list(range(N_CORES)))
    dev = np.stack([res.results[c]["out"] for c in range(N_CORES)])  # [C, EPC, WAY, 150]
    dev = dev.reshape(B, WAY, 2 * NQ)

    dn4 = dev[:, :, :NQ].transpose(0, 2, 1) / k                   # [B, nq, way]
    glob = dev[:, :, NQ:].transpose(0, 2, 1) / (HW * qmn[:, :, None])
    return (r[0] * glob + r[1] * dn4).astype(np.float32)
